# revision 10
# baseline (speedup 1.0000x reference)
"""Trainium2 Bass kernel for nn_LocalPoolPointNet (gnn_message_passing).

Sharding strategy (hardcoded):
  - 8 NeuronCores = 4 batches x 2 z-halves of the 64^3 grid. Points are
    sharded to the core owning their voxel's z-half, so every segment_max
    is core-local (a voxel's points all live on exactly one core) and no
    collective is needed.
  - Within a core, 4 "streams" (8 z-slices each) are folded across the 128
    SBUF partitions: partition 32*q + ch holds channel ch of stream q.
    Matmuls use block-diagonal [128,128] stationary weights so all 4
    streams multiply in a single PE pass at full array width.
  - Points are binned by voxel into fixed-size windows (power-of-two slot
    class sizes, padded by duplicating a point of the same voxel, which is
    max-neutral). segment_max becomes a windowed DVE tensor_reduce(max)
    and the gather-back is a stride-0 access-pattern broadcast consumed
    directly by the PE as the moving matmul operand. Single-point voxels
    (the majority) skip pooling entirely via pre-summed weight blocks.
  - v2 pipeline: block0 folds the residual-skip path into a 3-row weight
    (p @ (W_pos@Ws0)); block4 folds W_c into its weights so the final
    linear layer and net5 materialization disappear; elementwise work is
    spread across Scalar/DVE/Pool; PSUM tiles span 2 banks so vector ops
    run at 1024-col granularity; matmuls are weight-grouped to cut
    LDWEIGHTS; the table transpose + sparse scatter is interleaved with
    block4's sweep. Output grid zero-fill overlaps the MLP phase.
"""

import os
import sys

sys.path.insert(0, "/opt/trn_rl_repo")

import numpy as np

R = 64
B = 4
N = 100000
NB = 5
NCORES = 8
NSTREAM = 4
VOX_PER_STREAM = 64 * 64 * 8  # 32768
CHUNK = 512
CLASS_SIZES = (1, 2, 4, 8, 16, 32, 64, 128, 256, 512)


def _pick_bf16(L):
    return True


def _coords2index_np(p):
    """Exact float32 replica of reference._coords2index."""
    p = np.asarray(p, dtype=np.float32)
    pn = np.clip(p + np.float32(1.0), np.float32(0.0), np.float32(2.0 - 0.0001))
    xi = ((pn / np.float32(2.0)) * np.float32(R)).astype(np.int32)
    return xi[..., 0] + R * (xi[..., 1] + R * xi[..., 2])


def _class_of(occ):
    for k in CLASS_SIZES:
        if occ <= k:
            return k
    raise ValueError(f"voxel occupancy {occ} too large")


def _bin_streams(p_all):
    """Host-side sharding: bin points by (core, stream, voxel)."""
    idx_all = _coords2index_np(p_all)  # [B, N]
    binned = []
    for b in range(B):
        idx_b = idx_all[b]
        z = idx_b >> 12
        for h in range(2):
            streams = []
            for q in range(NSTREAM):
                z0 = 32 * h + 8 * q
                sel = np.nonzero((z >= z0) & (z < z0 + 8))[0]
                vloc = idx_b[sel] - 4096 * z0
                order = np.argsort(vloc, kind="stable")
                sel, vloc = sel[order], vloc[order]
                uvox, starts, counts = np.unique(
                    vloc, return_index=True, return_counts=True)
                by_class = {}
                for ui in range(len(uvox)):
                    by_class.setdefault(_class_of(counts[ui]), []).append(ui)
                streams.append(dict(sel=sel, uvox=uvox, starts=starts,
                                    counts=counts, by_class=by_class))
            binned.append(streams)
    return binned


def _build_layout(binned):
    """Cross-core/stream padded class layout.

    Returns [(k, nwin, wins_per_chunk)], slot total L, window total V.
    Each class region is a whole number of 512-col chunks."""
    classes = sorted({k for cs in binned for s in cs for k in s["by_class"]})
    layout = []
    for k in classes:
        wpc = CHUNK // k
        nw = max(len(s["by_class"].get(k, ())) for cs in binned for s in cs)
        nw = -(-nw // wpc) * wpc
        layout.append((k, nw, wpc))
    L = sum(k * nw for k, nw, _ in layout)
    V = sum(nw for _, nw, _ in layout)
    return layout, L, V


def _build_core_inputs(p_all, binned, layout, L, V):
    assert V + 1 <= 32768
    cores = []
    for core in range(NCORES):
        b, h = divmod(core, 2)
        p_f4 = np.zeros((12, L), dtype=np.float32)
        rank_map = np.full((NSTREAM, VOX_PER_STREAM), V, dtype=np.int32)
        for q in range(NSTREAM):
            s = binned[core][q]
            sel, uvox = s["sel"], s["uvox"]
            starts, counts = s["starts"], s["counts"]
            assert len(sel) > 0
            slot_pts = np.full(L, sel[0], dtype=np.int64)
            off_slot = 0
            off_win = 0
            for k, nw, _ in layout:
                for wi, ui in enumerate(s["by_class"].get(k, ())):
                    st, ct = starts[ui], counts[ui]
                    pts = sel[st:st + ct]
                    sl = off_slot + wi * k
                    slot_pts[sl:sl + k] = pts[0]
                    slot_pts[sl:sl + ct] = pts
                    rank_map[q, uvox[ui]] = off_win + wi
                off_slot += k * nw
                off_win += nw
            p_f4[3 * q:3 * q + 3, :] = p_all[b, slot_pts, :].T
        # inverse rank map for the sparse scatter: per (stream, window-tile)
        # column of row offsets into the [131072, 32] output (4*vloc + q),
        # pad windows get a huge offset (skipped via bounds_check)
        ntiles = -(-V // 128)
        voxoff = np.full((128, NSTREAM * ntiles), 1 << 20, dtype=np.int32)
        for q in range(NSTREAM):
            inv = np.full(V, 1 << 20, dtype=np.int64)
            occ = rank_map[q] < V
            vloc = np.nonzero(occ)[0]
            inv[rank_map[q][vloc]] = 4 * vloc + q
            for t in range(ntiles):
                w = inv[128 * t:128 * (t + 1)]
                voxoff[:len(w), q * ntiles + t] = w
        cores.append(dict(p_f4=p_f4, voxoff=voxoff))
    return cores


def _bd4(w):
    out = np.zeros((128, 128), dtype=np.float32)
    for q in range(4):
        out[32 * q:32 * q + 32, 32 * q:32 * q + 32] = w
    return out


def _bias_f4(bvec):
    return np.tile(np.asarray(bvec, np.float32), 4).reshape(128, 1)


def _build_weights(inp):
    W = {}
    W_pos = np.asarray(inp["W_pos"], np.float32)
    for half, sl in (("lo", slice(0, 32)), ("hi", slice(32, 64))):
        w = np.zeros((12, 128), dtype=np.float32)
        for q in range(4):
            w[3 * q:3 * q + 3, 32 * q:32 * q + 32] = W_pos[:, sl]
        W[f"wpos_{half}"] = w
    W["bpos_lo"] = _bias_f4(np.asarray(inp["b_pos"], np.float32)[:32])
    W["bpos_hi"] = _bias_f4(np.asarray(inp["b_pos"], np.float32)[32:])
    W0 = np.asarray(inp["W0"], np.float32)
    W1 = np.asarray(inp["W1"], np.float32)
    Ws = np.asarray(inp["Ws"], np.float32)
    Wc = np.asarray(inp["W_c"], np.float32)
    bc = np.asarray(inp["b_c"], np.float32)
    b_pos = np.asarray(inp["b_pos"], np.float32)
    # block0: skip path fused into a 3-row weight (p @ (W_pos @ Ws0));
    # the b_pos @ Ws0 part goes into b1_0
    wsk = W_pos @ Ws[0]  # [3, 32]
    w = np.zeros((12, 128), dtype=np.float32)
    for q in range(4):
        w[3 * q:3 * q + 3, 32 * q:32 * q + 32] = wsk
    W["wskip0"] = w
    for i in range(NB):
        W[f"w0a_{i}"] = _bd4(W0[i, :32])
        W[f"w0b_{i}"] = _bd4(W0[i, 32:])
        W[f"w0ab_{i}"] = _bd4(W0[i, :32] + W0[i, 32:])
        W[f"b0_{i}"] = _bias_f4(inp["b0"][i])
    for i in range(NB - 1):
        W[f"w1_{i}"] = _bd4(W1[i])
        W[f"wsa_{i}"] = _bd4(Ws[i, :32])
        W[f"wsb_{i}"] = _bd4(Ws[i, 32:])
        W[f"wsab_{i}"] = _bd4(Ws[i, :32] + Ws[i, 32:])
        b1 = np.asarray(inp["b1"][i], np.float32)
        if i == 0:
            b1 = b1 + b_pos @ Ws[0]
        W[f"b1_{i}"] = _bias_f4(b1)
    # block4: W_c folded into the last resnet block's output weights
    W["w1c_4"] = _bd4(W1[4] @ Wc)
    W["wsac_4"] = _bd4(Ws[4, :32] @ Wc)
    W["wsbc_4"] = _bd4(Ws[4, 32:] @ Wc)
    W["wsabc_4"] = _bd4((Ws[4, :32] + Ws[4, 32:]) @ Wc)
    W["bcf"] = _bias_f4(np.asarray(inp["b1"][4], np.float32) @ Wc + bc)
    return W


WNAMES = (["wpos_lo", "wpos_hi", "wskip0",
           "w1c_4", "wsac_4", "wsbc_4", "wsabc_4"]
          + [f"w0a_{i}" for i in range(NB)]
          + [f"w0b_{i}" for i in range(NB)]
          + [f"w0ab_{i}" for i in range(1, NB)]
          + [f"{nm}_{i}" for i in range(1, NB - 1)
             for nm in ("w1", "wsa", "wsb", "wsab")]
          + ["w1_0"])
BNAMES = (["bpos_lo", "bpos_hi", "bcf"]
          + [f"b0_{i}" for i in range(NB)]
          + [f"b1_{i}" for i in range(NB - 1)])


def _emit_program(layout, L, V, MM_BF16=True):
    from concourse import bacc, bass, mybir, tile
    from concourse.masks import make_identity

    f32 = mybir.dt.float32
    bf16 = mybir.dt.bfloat16
    add = mybir.AluOpType.add
    amax = mybir.AluOpType.max
    relu = mybir.ActivationFunctionType.Relu
    ident_fn = mybir.ActivationFunctionType.Identity

    assert L % CHUNK == 0
    nchunks = L // CHUNK

    nc = bacc.Bacc("TRN2", target_bir_lowering=False, debug=False,
                   num_devices=NCORES)

    d_p = nc.dram_tensor("p_f4", [12, L], bf16, kind="ExternalInput")
    ntiles = -(-V // 128)
    d_voxoff = nc.dram_tensor("voxoff", [128, NSTREAM * ntiles],
                              mybir.dt.int32, kind="ExternalInput")
    d_w = {}
    for nm in WNAMES:
        shape = [12, 128] if (nm.startswith("wpos") or nm == "wskip0") \
            else [128, 128]
        d_w[nm] = nc.dram_tensor(nm, shape, bf16, kind="ExternalInput")
    for nm in BNAMES:
        d_w[nm] = nc.dram_tensor(nm, [128, 1], f32, kind="ExternalInput")
    d_out = nc.dram_tensor("grid", [4 * VOX_PER_STREAM, 32], f32,
                           kind="ExternalOutput")

    # chunk bookkeeping (all chunks exactly CHUNK cols)
    chunk_info = []
    class_off = {}
    off_slot = 0
    off_win = 0
    for k, nw, wpc in layout:
        class_off[k] = (off_slot, off_win, nw)
        for c in range(nw // wpc):
            chunk_info.append(dict(
                k=k, slot0=off_slot + c * wpc * k, win0=off_win + c * wpc,
                nwin=wpc))
        off_slot += nw * k
        off_win += nw
    assert len(chunk_info) == nchunks
    # tiles = pairs of consecutive chunks
    tiles = [tuple(range(t, min(t + 2, nchunks)))
             for t in range(0, nchunks, 2)]
    k_of = [ci["k"] for ci in chunk_info]

    with tile.TileContext(nc) as tc:
        with tc.tile_pool(name="persist", bufs=1) as pers, \
             tc.tile_pool(name="chunks", bufs=3) as chp, \
             tc.tile_pool(name="mpool", bufs=2) as mp, \
             tc.tile_pool(name="gridp", bufs=2) as grp:

            sb_w = {}
            for nm in WNAMES:
                shape = [12, 128] if (nm.startswith("wpos") or nm == "wskip0") \
                    else [128, 128]
                t = pers.tile(shape, bf16, tag=nm)
                nc.sync.dma_start(out=t[:], in_=d_w[nm][:])
                sb_w[nm] = t
            for nm in BNAMES:
                t = pers.tile([128, 1], f32, tag=nm)
                nc.sync.dma_start(out=t[:], in_=d_w[nm][:])
                sb_w[nm] = t
            voxoff_sb = pers.tile([128, NSTREAM * ntiles], mybir.dt.int32,
                                  tag="voxoff")
            nc.sync.dma_start(out=voxoff_sb[:], in_=d_voxoff[:])
            ident = pers.tile([128, 128], f32, tag="ident")
            make_identity(nc, ident[:])
            p_full = pers.tile([12, L], bf16, tag="p_full")
            nc.sync.dma_start(out=p_full[:], in_=d_p[:])

            zt = pers.tile([128, 1024], f32, tag="zt")
            nc.vector.memset(zt[:], 0.0)
            zview = d_out[:].rearrange("(p a) c -> p a c", p=128)
            zfill = [lambda j=j: nc.gpsimd.dma_start(
                out=zview[:, 32 * j:32 * (j + 1), :],
                in_=zt[:].rearrange("p (a c) -> p a c", c=32))
                for j in range(32)]
            zfill_i = 0

            def emit_zfill(n):
                nonlocal zfill_i
                for _ in range(n):
                    if zfill_i < len(zfill):
                        zfill[zfill_i]()
                        zfill_i += 1

            bigA = pers.tile([128, L], bf16, tag="bigA")
            bigB = pers.tile([128, L], bf16, tag="bigB")
            table = pers.tile([128, 128 * ntiles], f32, tag="table")

            def mm(ps, wname, rhs, start, stop):
                nc.tensor.matmul(out=ps, lhsT=sb_w[wname][:], rhs=rhs,
                                 start=start, stop=stop)

            # engine-rotating helpers for PSUM->SBUF evictions
            def ev_relu_bias(eng, out, in_, bap):
                if eng == "S":
                    nc.scalar.activation(out, in_, relu, bias=bap)
                else:
                    e = nc.vector if eng == "D" else nc.gpsimd
                    e.tensor_scalar(out=out, in0=in_, scalar1=bap,
                                    scalar2=0.0, op0=add, op1=amax)

            def ev_add_bias(eng, out, in_, bap):
                if eng == "S":
                    nc.scalar.activation(out, in_, ident_fn, bias=bap)
                else:
                    e = nc.vector if eng == "D" else nc.gpsimd
                    e.tensor_scalar_add(out=out, in0=in_, scalar1=bap)

            # M/RM pooling tiles for the *next* block, written per-chunk as
            # the current block's output is evicted
            def new_m_tiles(i):
                M, RM = {}, {}
                for k, nw, wpc in layout:
                    if k == 1:
                        continue
                    M[k] = mp.tile([128, nw], bf16, tag=f"M_{k}",
                                   name=f"M_{k}")
                    RM[k] = mp.tile([128, nw], bf16, tag=f"RM_{k}",
                                    name=f"RM_{k}")
                return M, RM

            def emit_reduce(M, RM, ci, src):
                """windowed max of chunk ci from bf16 SBUF src -> M/RM."""
                info = chunk_info[ci]
                k, s0, w0, nwin = info["k"], info["slot0"], info["win0"], \
                    info["nwin"]
                if k == 1:
                    return
                wrel = w0 - class_off[k][1]
                nc.vector.tensor_reduce(
                    out=M[k][:, wrel:wrel + nwin],
                    in_=src[:, s0:s0 + CHUNK]
                    .rearrange("p (w k) -> p w k", k=k),
                    axis=mybir.AxisListType.X, op=amax)
                nc.vector.tensor_scalar_max(
                    out=RM[k][:, wrel:wrel + nwin],
                    in0=M[k][:, wrel:wrel + nwin], scalar1=0.0)

            # ================= block 0 =================
            M, RM = new_m_tiles(1)
            with tc.tile_pool(name="ps0", bufs=2, space="PSUM") as ps0:
                for ti, tch in enumerate(tiles):
                    for j, ci in enumerate(tch):
                        s0 = chunk_info[ci]["slot0"]
                        psP = ps0.tile([128, 1024], f32, space="PSUM",
                                       tag="psP")
                        pch = p_full[:, s0:s0 + CHUNK]
                        mm(psP[:, 0:512], "wpos_lo", pch, True, True)
                        mm(psP[:, 512:1024], "wpos_hi", pch, True, True)
                        rlo = chp.tile([128, 512], bf16, tag="rlo")
                        rhi = chp.tile([128, 512], bf16, tag="rhi")
                        ev_relu_bias("S", rlo[:], psP[:, 0:512],
                                     sb_w["bpos_lo"][:, 0:1])
                        ev_relu_bias("S" if ci % 5 == 0 else "D", rhi[:],
                                     psP[:, 512:1024],
                                     sb_w["bpos_hi"][:, 0:1])
                        psA = ps0.tile([128, 512], f32, space="PSUM",
                                       tag="psA")
                        mm(psA[:], "w0a_0", rlo[:], True, False)
                        mm(psA[:], "w0b_0", rhi[:], False, True)
                        ra = chp.tile([128, 512], bf16, tag="ra0")
                        ev_relu_bias("S", ra[:], psA[:],
                                     sb_w["b0_0"][:, 0:1])
                        psD = ps0.tile([128, 512], f32, space="PSUM",
                                       tag="psD")
                        mm(psD[:], "w1_0", ra[:], True, False)
                        mm(psD[:], "wskip0", pch, False, True)
                        ev_add_bias("D", bigA[:, s0:s0 + CHUNK], psD[:],
                                    sb_w["b1_0"][:, 0:1])
                        emit_reduce(M, RM, ci, bigA[:])
                    if ti % 2 == 0:
                        emit_zfill(1)

            # ================= blocks 1..4 =================
            cur, nxt = bigA, bigB
            wdone = 0  # transposed window-tiles emitted
            win_ready = 0

            def emit_transposes(limit, psp):
                """transpose+stage+scatter all ready 128-window tiles."""
                nonlocal wdone
                while wdone < limit:
                    g0 = wdone
                    gn = min(4, limit - g0)
                    tt = grp.tile([128, 512], f32, tag="tt")
                    psT = psp.tile([128, 1024], f32, space="PSUM", tag="psD")
                    for g in range(gn):
                        j = g0 + g
                        w = min(128, V - 128 * j)
                        nc.tensor.transpose(
                            out=psT[:w, 128 * g:128 * (g + 1)],
                            in_=table[:, 128 * j:128 * j + w],
                            identity=ident[:])
                        if j % 2 == 0:
                            nc.vector.tensor_copy(
                                out=tt[:w, 128 * g:128 * (g + 1)],
                                in_=psT[:w, 128 * g:128 * (g + 1)])
                        else:
                            nc.scalar.copy(
                                out=tt[:w, 128 * g:128 * (g + 1)],
                                in_=psT[:w, 128 * g:128 * (g + 1)])
                    for g in range(gn):
                        j = g0 + g
                        for q in range(NSTREAM):
                            nc.gpsimd.indirect_dma_start(
                                out=d_out[:],
                                out_offset=bass.IndirectOffsetOnAxis(
                                    ap=voxoff_sb[:, q * ntiles + j:
                                                 q * ntiles + j + 1], axis=0),
                                in_=tt[:, 128 * g + 32 * q:
                                       128 * g + 32 * q + 32],
                                in_offset=None,
                                bounds_check=4 * VOX_PER_STREAM - 1,
                                oob_is_err=False)
                    wdone += gn

            with tc.tile_pool(name="ps1", bufs=2, space="PSUM") as ps1:
                for i in range(1, NB):
                    last = i == NB - 1
                    Mn, RMn = (None, None) if last else new_m_tiles(i + 1)
                    sfx = "c_4" if last else f"_{i}"
                    for ti, tch in enumerate(tiles):
                        t0 = chunk_info[tch[0]]["slot0"]
                        tcols = CHUNK * len(tch)
                        # rn for this tile (Pool, SBUF->SBUF)
                        rn = chp.tile([128, 1024], bf16, tag="rn")
                        nc.gpsimd.tensor_scalar_max(
                            out=rn[:, 0:tcols], in0=cur[:, t0:t0 + tcols],
                            scalar1=0.0)
                        psA = ps1.tile([128, 1024], f32, space="PSUM",
                                       tag="psA")
                        # --- psA: weight-grouped across the tile's chunks
                        for j, ci in enumerate(tch):
                            if k_of[ci] == 1:
                                mm(psA[:, 512 * j:512 * (j + 1)], f"w0ab_{i}",
                                   rn[:, 512 * j:512 * j + 512], True, True)
                        for j, ci in enumerate(tch):
                            if k_of[ci] > 1:
                                mm(psA[:, 512 * j:512 * (j + 1)], f"w0a_{i}",
                                   rn[:, 512 * j:512 * j + 512], True, False)
                        for j, ci in enumerate(tch):
                            k = k_of[ci]
                            if k > 1:
                                info = chunk_info[ci]
                                wrel = info["win0"] - class_off[k][1]
                                bc = RM[k][:, wrel:wrel + info["nwin"]] \
                                    .unsqueeze(2) \
                                    .to_broadcast([128, info["nwin"], k])
                                mm(psA[:, 512 * j:512 * (j + 1)], f"w0b_{i}",
                                   bc, False, True)
                        ra = chp.tile([128, 1024], bf16, tag="ra")
                        ev_relu_bias("S", ra[:, 0:tcols], psA[:, 0:tcols],
                                     sb_w[f"b0_{i}"][:, 0:1])
                        psD = ps1.tile([128, 1024], f32, space="PSUM",
                                       tag="psD")
                        # --- psD: w1 then skip weights, weight-grouped
                        for j, ci in enumerate(tch):
                            mm(psD[:, 512 * j:512 * (j + 1)], f"w1{sfx}",
                               ra[:, 512 * j:512 * j + 512], True, False)
                        for j, ci in enumerate(tch):
                            if k_of[ci] == 1:
                                mm(psD[:, 512 * j:512 * (j + 1)],
                                   f"wsab{sfx}",
                                   cur[:, t0 + 512 * j:t0 + 512 * j + 512],
                                   False, True)
                        for j, ci in enumerate(tch):
                            if k_of[ci] > 1:
                                mm(psD[:, 512 * j:512 * (j + 1)],
                                   f"wsa{sfx}",
                                   cur[:, t0 + 512 * j:t0 + 512 * j + 512],
                                   False, False)
                        for j, ci in enumerate(tch):
                            k = k_of[ci]
                            if k > 1:
                                info = chunk_info[ci]
                                wrel = info["win0"] - class_off[k][1]
                                bc = M[k][:, wrel:wrel + info["nwin"]] \
                                    .unsqueeze(2) \
                                    .to_broadcast([128, info["nwin"], k])
                                mm(psD[:, 512 * j:512 * (j + 1)],
                                   f"wsb{sfx}", bc, False, True)
                        if not last:
                            # evict net' (+b1) to nxt, engine-rotating
                            eng = "SD"[ti % 2]
                            ev_add_bias(eng, nxt[:, t0:t0 + tcols],
                                        psD[:, 0:tcols],
                                        sb_w[f"b1_{i}"][:, 0:1])
                            for ci in tch:
                                emit_reduce(Mn, RMn, ci, nxt[:])
                            if i < 3:
                                emit_zfill(1)
                        else:
                            # block4: psD holds c (pre-bias); fold into table
                            for j, ci in enumerate(tch):
                                info = chunk_info[ci]
                                k, w0, nwin = info["k"], info["win0"], \
                                    info["nwin"]
                                src = psD[:, 512 * j:512 * (j + 1)]
                                if k == 1:
                                    eng = "SD"[ci % 2]
                                    ev_relu_bias(
                                        eng, table[:, w0:w0 + nwin], src,
                                        sb_w["bcf"][:, 0:1])
                                else:
                                    mc = chp.tile([128, 256], f32, tag="mc")
                                    nc.vector.tensor_reduce(
                                        out=mc[:, 0:nwin],
                                        in_=src.rearrange(
                                            "p (w k) -> p w k", k=k),
                                        axis=mybir.AxisListType.X, op=amax)
                                    ev_relu_bias(
                                        "D", table[:, w0:w0 + nwin],
                                        mc[:, 0:nwin], sb_w["bcf"][:, 0:1])
                                win_ready = w0 + nwin
                            # emit transposes for fully-ready window tiles,
                            # one group behind to decouple from psD reuse
                            emit_transposes(
                                max(0, min(win_ready // 128 - 4, wdone + 4)),
                                ps1)
                    if not last:
                        M, RM = Mn, RMn
                        cur, nxt = nxt, cur
                emit_transposes(ntiles, ps1)

    nc.compile()
    return nc


_CACHE = {}


def _to_mm_dtype(arr, MM_BF16=True):
    import ml_dtypes
    return np.asarray(arr, np.float32).astype(ml_dtypes.bfloat16)


def kernel(**inputs):
    from concourse.bass_utils import run_bass_kernel_spmd

    p_all = np.asarray(inputs["p"], np.float32)
    binned = _bin_streams(p_all)
    layout, L, V = _build_layout(binned)
    cores = _build_core_inputs(p_all, binned, layout, L, V)
    W = _build_weights(inputs)

    key = (tuple(layout), L, V)
    if key not in _CACHE:
        _CACHE[key] = _emit_program(layout, L, V)
    nc = _CACHE[key]

    in_maps = []
    for core in range(NCORES):
        m = {"p_f4": _to_mm_dtype(cores[core]["p_f4"]),
             "voxoff": cores[core]["voxoff"]}
        for nm in WNAMES:
            m[nm] = _to_mm_dtype(W[nm])
        for nm in BNAMES:
            m[nm] = W[nm].astype(np.float32)
        in_maps.append(m)

    res = run_bass_kernel_spmd(nc, in_maps, list(range(NCORES)))

    out = np.zeros((B, 32, R, R, R), dtype=np.float32)
    for core in range(NCORES):
        b, h = divmod(core, 2)
        g = res.results[core]["grid"]  # [131072, 32] = (vloc, stream, ch)
        g = g.reshape(32768, 4, 32).transpose(1, 2, 0)  # [q, ch, vloc]
        g = g.reshape(4, 32, 8, 64, 64).transpose(1, 0, 2, 3, 4)
        out[b, :, 32 * h:32 * h + 32] = g.reshape(32, 32, 64, 64)
    return out
